# revision 46
# baseline (speedup 1.0000x reference)
"""AlphaNet forward pass on 8 Trainium2 NeuronCores (data-parallel over batch).

Pipeline per core (512 samples):
  DVE: rolling-window stats (corr/cov/std/zscore/return/decaylinear) in an
       unscaled "mine" form, written into a padded [272 rows x 16 win] bf16
       buffer. Mine row order: corr(0:105) cov(105:210) std(210:225)
       zs(225:240) dl(240:255) dummy(255) ret(256:271) -- ret rows last so
       the fp8 fc1 region excludes the huge-dynamic-range return rows,
       which stay in a bf16 tail.
  PE : transpose features to [rows, samples] (bf16); conv(1x3)+BN folded
       into per-tile A matmuls; fc1 as fp8-e4m3 DoubleRow matmuls (K=256
       per instruction) plus a bf16 tail; fc2/fc3 bf16. fc1 weights are
       pre-scaled by GQ=4096 into e4m3's normal range; the fc1 relu
       epilogue divides by GQ.
  Overlap: stats are computed pair-offset-major (d=1..14 across all four
       sample blocks), and conv/fc1 tile pairs are issued in row-readiness
       order so the PE stream starts right after the cheap simple stats
       and runs concurrently with the remaining DVE work. Scalar engine
       owns the conv epilogues (bias+relu -> fp8/bf16); GpSimd takes
       memsets and the d=1 cov products. fc1 matmuls for pair k are issued
       after the conv matmuls of pair k+1 so PE never stalls on epilogues.
All per-row constant factors (BatchNorm affine, 1/9, 0.9, 0.3, ret's -1, ...)
are folded into the host-built conv matrix A and per-row bias. Weights are
host-permuted/packed in issue order; DMAs are batched (16 A-tiles / 8 fc1
weight pairs per transfer) from partition-major DRAM layouts.
"""
import sys
for _p in ("/opt/trn_rl_repo", "/root/.axon_site/_ro/trn_rl_repo"):
    if _p not in sys.path:
        sys.path.append(_p)

from contextlib import ExitStack

import numpy as np
import ml_dtypes

import concourse.bass as bass
import concourse.tile as tile
from concourse import bacc, mybir
from concourse.bass_utils import run_bass_kernel_spmd
from concourse.masks import make_identity

bf16 = ml_dtypes.bfloat16
f8e4 = ml_dtypes.float8_e4m3
dt = mybir.dt

# ---- problem constants (hardcoded; must match the AlphaNet reference) ----
NFULL = 4096
NCORES = 8
NSH = NFULL // NCORES        # 512 samples per core
F, W, S = 15, 120, 10
NW = W // S                  # 12
HP = 270                     # reference stat rows
HPM = 271                    # mine rows incl dummy row 255
NROW_PAD, WPAD = 272, 16
GROWS = NROW_PAD * WPAD      # 4352 = 34*128
NGT = GROWS // 128           # 34 transposed-feature tiles
K1M = HPM * 160              # 43360 kernel-order K
NT = 339                     # ceil(K1M/128)
K1PAD = NT * 128             # 43392
NT8 = 320                    # fp8 tiles (pairs 0:160); bf16 tail 320:339
NPAIR = NT8 // 2             # 160 DoubleRow pairs
NTAIL = NT - NT8             # 19
NTA = 352                    # A tile slots padded to 22*16 for uniform batches
BN_EPS = 1e-5
NB = NSH // 128              # 4 sample blocks per core
GQ = 4096.0                  # global fc1 weight scale (fp8 normal range)
GPSIMD_DD = ()               # cov products on GpSimd contend for DVE's SBUF ports
NLVL = 18                    # readiness levels: 0=std 1=ret 2=zs 3=dl 3+d=cov/corr d
EPI_DVE_FROM = 88            # items from this index split epilogues Scalar/DVE

# mine row blocks
R_CORR, R_COV, R_STD, R_ZS, R_DL, R_DUM, R_RET = 0, 105, 210, 225, 240, 255, 256


# ------------------------- host-side preparation -------------------------

def _mine_row_tables():
    cb, pairs, base = {}, [], 0
    for d in range(1, 15):
        cb[d] = base
        for i in range(0, 15 - d):
            pairs.append((i, i + d))
        base += 15 - d
    return pairs, cb


def _ref_perm():
    """rom[mine_row] = reference feature row (or -1 for the dummy row)."""
    pairs, _ = _mine_row_tables()
    II, JJ = np.triu_indices(F, k=1)
    p2r = {(int(i), int(j)): p for p, (i, j) in enumerate(zip(II, JJ))}
    rom = np.full(HPM, -1, dtype=np.int64)
    for mh, (i, j) in enumerate(pairs):
        rom[mh] = p2r[(i, j)]
        rom[R_COV + mh] = 105 + p2r[(i, j)]
    for i in range(15):
        rom[R_STD + i] = 210 + i
        rom[R_ZS + i] = 225 + i
        rom[R_DL + i] = 255 + i     # reference decaylinear block
        rom[R_RET + i] = 240 + i    # reference return block
    return rom


def _row_alpha_beta():
    """mine = alpha*ref + beta per row."""
    alpha = np.ones(HPM)
    beta = np.zeros(HPM)
    alpha[R_CORR:R_COV] = 1.0 / 0.9
    alpha[R_COV:R_STD] = 9.0
    alpha[R_STD:R_ZS] = 3.0
    alpha[R_ZS:R_DL] = 10.0 / 3.0
    alpha[R_DL:R_DUM] = 1.0
    alpha[R_RET:] = 1.0
    beta[R_RET:] = 1.0
    return alpha, beta


def _piece_refs():
    refs = []
    for t in range(NT):
        h0 = (128 * t) // 160
        hl = min((128 * t + 127) // 160, HPM - 1)
        u = h0 // 8
        assert all(8 * u <= h < 8 * u + 8 for h in range(h0, hl + 1)), (t, h0, hl)
        refs.append(u)
    return refs


def _schedule(refs):
    """Row-readiness schedule. Returns (ulvl, items) where items is the
    conv/fc1 issue order: dicts {kind, lvl, tiles, pslot or tidx, aslots}.
    lvl is the stats level after which the item's rows are complete:
    0=std 1=zscore 2=decaylinear 3=return 3+d=cov/corr offset d."""
    _, cb = _mine_row_tables()

    def lvl_row(r):
        if r >= R_RET:
            return 0        # return rows unlock the whole bf16 tail first
        if R_STD <= r < R_ZS:
            return 1
        if R_ZS <= r < R_DL:
            return 2
        if R_DL <= r < R_DUM:
            return 3
        c = r if r < 105 else r - 105
        for d in range(1, 15):
            if c < cb[d] + (15 - d):
                return 3 + d
        raise AssertionError(r)

    ulvl = []
    for u in range(NGT):
        m = 0
        for r in range(8 * u, 8 * u + 8):
            if r < HPM and r != R_DUM:
                m = max(m, lvl_row(r))
        ulvl.append(m)

    pair_lvl = [max(ulvl[refs[2 * P]], ulvl[refs[2 * P + 1]])
                for P in range(NPAIR)]
    items = []
    for lvl in range(NLVL):
        for P in range(NPAIR):
            if pair_lvl[P] == lvl:
                items.append(dict(kind='pair', lvl=lvl, P=P,
                                  tiles=[2 * P, 2 * P + 1]))
        if lvl == 0:
            for t in range(NT8, NT):
                assert ulvl[refs[t]] == 0
                items.append(dict(kind='tail', lvl=0, tiles=[t],
                                  tidx=t - NT8))
    # assign A slots and fp8 pair slots in issue order
    aslot = 0
    pslot = 0
    for it in items:
        it['aslots'] = list(range(aslot, aslot + len(it['tiles'])))
        aslot += len(it['tiles'])
        if it['kind'] == 'pair':
            it['pslot'] = pslot
            pslot += 1
    assert aslot == NT and pslot == NPAIR
    return ulvl, items


def _build_device_inputs(inp):
    gamma = float(inp['bn_gamma'][0]); betab = float(inp['bn_beta'][0])
    mu = float(inp['bn_mean'][0]); var = float(inp['bn_var'][0])
    a = gamma / np.sqrt(var + BN_EPS)
    b = betab - mu * a
    conv_w = np.asarray(inp['conv_w'], np.float64).reshape(16, 3)
    conv_b = np.asarray(inp['conv_b'], np.float64)

    alpha, beta = _row_alpha_beta()
    sA = a / alpha
    sB = b - a * beta / alpha

    refs = _piece_refs()
    ulvl, items = _schedule(refs)

    wsum = conv_w.sum(axis=1)
    ybias = np.zeros(K1PAD, np.float64)
    for mh in range(HPM):
        if mh == R_DUM:
            continue
        ybias[mh * 160:(mh + 1) * 160] = np.repeat(conv_b + wsum * sB[mh], 10)

    # fc1 weights: permute reference K to kernel (mh, o, wp) order, scale GQ
    rom = _ref_perm()
    m = np.arange(K1M)
    mh = m // 160; o = (m % 160) // 10; wp = m % 10
    valid = mh != R_DUM
    kref = o * 2700 + rom[mh] * 10 + wp
    fc1_w = np.asarray(inp['fc1_w'], np.float32)
    fc1t = np.zeros((K1PAD, 512), np.float32)
    fc1t[m[valid], :] = fc1_w[:, kref[valid]].T * GQ

    # conv A per tile [t, 128(k), 128(col)]
    A_t = np.zeros((NT, 128, 128), np.float32)
    for t in range(NT):
        u = refs[t]
        for col in range(128):
            mm = 128 * t + col
            if mm >= K1M:
                continue
            mhh = mm // 160
            if mhh == R_DUM:
                continue
            oo = (mm % 160) // 10
            wpp = mm % 10
            for k in range(3):
                A_t[t, (mhh - 8 * u) * 16 + wpp + k, col] = conv_w[oo, k] * sA[mhh]

    # pack A / ybias by A slot and fc1 fp8 pairs by pair slot (issue order)
    A8 = np.zeros((128, NTA, 128), np.float32)
    yb2d = np.zeros((128, NTA), np.float32)
    w8 = np.zeros((128, NPAIR, 2, 512), np.float32)
    for it in items:
        for t, s in zip(it['tiles'], it['aslots']):
            A8[:, s, :] = A_t[t]
            yb2d[:, s] = ybias[128 * t:128 * (t + 1)]
        if it['kind'] == 'pair':
            for i in range(2):
                t = it['tiles'][i]
                w8[:, it['pslot'], i, :] = fc1t[128 * t:128 * (t + 1), :]
    assert np.abs(w8).max() < 200.0, np.abs(w8).max()
    A8 = A8.astype(bf16)
    w8 = w8.astype(f8e4)
    # bf16 tail weights by tidx: [128(k), NTAIL, 512]
    w1t = np.ascontiguousarray(
        fc1t[NT8 * 128:].reshape(NTAIL, 128, 512).transpose(1, 0, 2)).astype(bf16)

    fc1b2d = np.asarray(inp['fc1_b'], np.float32).reshape(4, 128).T.copy()
    fc2t = np.ascontiguousarray(
        np.asarray(inp['fc2_w'], np.float32).T).astype(bf16)
    fc2b = np.asarray(inp['fc2_b'], np.float32).reshape(128, 1).copy()
    fc3t = np.ascontiguousarray(
        np.asarray(inp['fc3_w'], np.float32).reshape(1, 128).T).astype(bf16)
    fc3b = np.asarray(inp['fc3_b'], np.float32).reshape(1, 1).copy()
    wdl = np.tile((np.arange(1, 11, dtype=np.float32) / 55.0)[None, :],
                  (128, 1)).astype(bf16)

    return dict(A8=A8, w8=w8, w1t=w1t, piece_refs=refs,
                ulvl=ulvl, items=items,
                ybias2d=yb2d, fc1b2d=fc1b2d,
                fc2t=fc2t, fc2b=fc2b, fc3t=fc3t, fc3b=fc3b, wdl=wdl)


# ------------------------- device kernel builder -------------------------

def build_nc(piece_refs, ulvl, items):
    nc = bacc.Bacc("TRN2", target_bir_lowering=False, debug=False,
                   num_devices=NCORES)
    f32, b16, f8 = dt.float32, dt.bfloat16, dt.float8e4
    data_e = nc.declare_dram_parameter("data", [NSH, F * W], b16, isOutput=False)
    data0_e = nc.declare_dram_parameter("data0", [NSH, F * NW], f32, isOutput=False)
    A_e = nc.declare_dram_parameter("A8", [128, NTA, 128], b16, isOutput=False)
    w8_e = nc.declare_dram_parameter("w8", [128, NPAIR, 2, 512], f8, isOutput=False)
    w1t_e = nc.declare_dram_parameter("w1t", [128, NTAIL, 512], b16, isOutput=False)
    yb_e = nc.declare_dram_parameter("ybias2d", [128, NTA], f32, isOutput=False)
    fc1b_e = nc.declare_dram_parameter("fc1b2d", [128, 4], f32, isOutput=False)
    fc2t_e = nc.declare_dram_parameter("fc2t", [512, 128], b16, isOutput=False)
    fc2b_e = nc.declare_dram_parameter("fc2b", [128, 1], f32, isOutput=False)
    fc3t_e = nc.declare_dram_parameter("fc3t", [128, 1], b16, isOutput=False)
    fc3b_e = nc.declare_dram_parameter("fc3b", [1, 1], f32, isOutput=False)
    wdl_e = nc.declare_dram_parameter("wdl", [128, 10], b16, isOutput=False)
    out_e = nc.declare_dram_parameter("out", [1, NSH], f32, isOutput=True)

    _, cb = _mine_row_tables()
    AF = mybir.ActivationFunctionType
    DR = mybir.MatmulPerfMode.DoubleRow

    with tile.TileContext(nc) as tc, ExitStack() as ctx:
        consts = ctx.enter_context(tc.tile_pool(name="consts", bufs=1))
        fpool = ctx.enter_context(tc.tile_pool(name="fpool", bufs=1))
        datap = ctx.enter_context(tc.tile_pool(name="datap", bufs=4))
        featp = ctx.enter_context(tc.tile_pool(name="featp", bufs=4))
        spreadp = ctx.enter_context(tc.tile_pool(name="spreadp", bufs=4))
        meansump = ctx.enter_context(tc.tile_pool(name="meansump", bufs=4))
        rstdp = ctx.enter_context(tc.tile_pool(name="rstdp", bufs=4))
        scratch = ctx.enter_context(tc.tile_pool(name="scratch", bufs=4))
        gscratch = ctx.enter_context(tc.tile_pool(name="gscratch", bufs=2))
        xtp8 = ctx.enter_context(tc.tile_pool(name="xtp8", bufs=4))
        xtbp = ctx.enter_context(tc.tile_pool(name="xtbp", bufs=4))
        w8pool = ctx.enter_context(tc.tile_pool(name="w8pool", bufs=3))
        apool = ctx.enter_context(tc.tile_pool(name="apool", bufs=3))
        x2pool = ctx.enter_context(tc.tile_pool(name="x2pool", bufs=1))
        outp = ctx.enter_context(tc.tile_pool(name="outp", bufs=1))
        ps_fc1 = ctx.enter_context(tc.tile_pool(name="ps_fc1", bufs=1, space="PSUM"))
        ps_conv = ctx.enter_context(tc.tile_pool(name="ps_conv", bufs=4, space="PSUM"))

        lp = ctx.enter_context(nc.allow_low_precision(
            reason="stat rows feed a bf16 feature buffer; DVE reduces "
                   "accumulate in f32 internally, only the store is bf16"))

        # data DMAs first so stats start immediately
        dtiles = []
        d0tiles = []
        for bkl in range(NB):
            d = datap.tile([128, F, NW, S], b16, tag="d", name=f"d{bkl}")
            nc.sync.dma_start(
                d[:], data_e[128 * bkl:128 * (bkl + 1), :]
                .rearrange("p (f nw s) -> p f nw s", f=F, nw=NW))
            dtiles.append(d)
            d0 = datap.tile([128, F, NW], f32, tag="d0", name=f"d0{bkl}")
            nc.sync.dma_start(
                d0[:], data0_e[128 * bkl:128 * (bkl + 1), :]
                .rearrange("p (f nw) -> p f nw", f=F))
            d0tiles.append(d0)

        # constants
        yb_sb = consts.tile([128, NTA], f32)
        nc.sync.dma_start(yb_sb[:], yb_e[:])
        fc1b_sb = consts.tile([128, 4], f32)
        nc.sync.dma_start(fc1b_sb[:], fc1b_e[:])
        fc2t_sb = consts.tile([128, 4, 128], b16)
        nc.sync.dma_start(fc2t_sb[:], fc2t_e.rearrange("(kb k) j -> k kb j", k=128))
        fc2b_sb = consts.tile([128, 1], f32)
        nc.sync.dma_start(fc2b_sb[:], fc2b_e[:])
        fc3t_sb = consts.tile([128, 1], b16)
        nc.sync.dma_start(fc3t_sb[:], fc3t_e[:])
        fc3b_sb = consts.tile([1, 1], f32)
        nc.sync.dma_start(fc3b_sb[:], fc3b_e[:])
        wdl_sb = consts.tile([128, 10], b16)
        nc.sync.dma_start(wdl_sb[:], wdl_e[:])
        w1t_sb = consts.tile([128, NTAIL, 512], b16)
        ident = consts.tile([128, 128], b16)
        make_identity(nc, ident[:])

        # dummy matmuls to burn through the HAM half-clock warmup window
        # while DVE computes the head stats (PE would otherwise start cold)
        for wu in range(32):
            tpw = ps_conv.tile([128, 128], b16, tag="cps", name=f"warm{wu}")
            nc.tensor.transpose(tpw[:], ident[:], ident[:])

        # persistent bf16 transposed-feature buffer [row, sample]
        f_sb = fpool.tile([128, NGT, NSH], b16)

        feats, spreads, rstds, meansums, varsums = [], [], [], [], []

        # ---------------- stats emission (DVE/GpSimd/Scalar queues) -------
        # Ordered so conv/fc1 work unlocks as early as possible: per-block
        # core (spread/var/std) first, then zscore, decaylinear, return,
        # then cov/corr offsets d=1..14. The conv/fc1 stream below is
        # emitted to independent engine queues; cross-engine semaphores
        # enforce the data dependencies.
        # padding memsets up front on GpSimd, while the data DMAs are in
        # flight (they only gate the transposes, not the stats)
        for bkl in range(NB):
            feat = featp.tile([128, NROW_PAD, WPAD], b16, tag="feat",
                              name=f"feat{bkl}")
            # zero only the padding (w cols 12:16, dummy row 255, pad row 271)
            nc.gpsimd.memset(feat[:, :, NW:WPAD], 0.0)
            nc.gpsimd.memset(feat[:, R_DUM:R_DUM + 1, 0:NW], 0.0)
            nc.gpsimd.memset(feat[:, HPM:NROW_PAD, 0:NW], 0.0)
            feats.append(feat)

        # level 0: return rows (last/first) straight off the raw data --
        # they unlock the whole bf16 tail, giving PE work while the std
        # chain below still runs. level 1: std via
        # sigma^2 = sum(d^2) - (sum d)^2/S, so neither mean/spread nor any
        # reciprocal gates the early tiles.
        with tc.high_priority():
            for bkl in range(NB):
                recipf = scratch.tile([128, F, NW], f32, tag="s180f")
                nc.vector.reciprocal_approx_fast(recipf[:], d0tiles[bkl][:])
                nc.vector.tensor_mul(feats[bkl][:, R_RET:R_RET + 15, 0:NW],
                                     dtiles[bkl][:, :, :, S - 1], recipf[:])
            for bkl in range(NB):
                d = dtiles[bkl]
                feat = feats[bkl]
                meansum = meansump.tile([128, F, NW], f32, tag="ms",
                                        name=f"ms{bkl}")
                nc.vector.tensor_reduce(meansum[:], d[:],
                                        axis=mybir.AxisListType.X,
                                        op=mybir.AluOpType.add)
                ddp = scratch.tile([128, F, NW, S], b16, tag="prod")
                nc.vector.tensor_mul(ddp[:], d[:], d[:])
                ddsum = scratch.tile([128, F, NW], f32, tag="s180f")
                nc.vector.tensor_reduce(ddsum[:], ddp[:],
                                        axis=mybir.AxisListType.X,
                                        op=mybir.AluOpType.add)
                msq = scratch.tile([128, F, NW], f32, tag="s180g")
                nc.vector.tensor_mul(msq[:], meansum[:], meansum[:])
                varsum = scratch.tile([128, F, NW], f32, tag="vsum")
                nc.vector.scalar_tensor_tensor(varsum[:], msq[:], -1.0 / S,
                                               ddsum[:],
                                               op0=mybir.AluOpType.mult,
                                               op1=mybir.AluOpType.add)
                nc.scalar.activation(feat[:, R_STD:R_STD + 15, 0:NW], varsum[:],
                                     AF.Sqrt, bias=0.0, scale=1.0)
                varsums.append(varsum)
                meansums.append(meansum)
        # zscore (level 2); rstd = sqrt(1/varsum), sqrt on Scalar
        for bkl in range(NB):
            rvar = scratch.tile([128, F, NW], f32, tag="s180g")
            nc.vector.reciprocal_approx_fast(rvar[:], varsums[bkl][:])
            rstd = rstdp.tile([128, F, NW], f32, tag="rstd", name=f"rstd{bkl}")
            nc.scalar.activation(rstd[:], rvar[:], AF.Sqrt,
                                 bias=0.0, scale=1.0)
            rstds.append(rstd)
            nc.vector.tensor_mul(feats[bkl][:, R_ZS:R_ZS + 15, 0:NW],
                                 meansums[bkl][:], rstd[:])
        # decay-linear (level 3, mine rows 240:255)
        for bkl in range(NB):
            dlp = scratch.tile([128, F, NW, S], b16, tag="prod")
            nc.vector.tensor_mul(
                dlp[:], dtiles[bkl][:],
                wdl_sb[:, None, None, :].to_broadcast((128, F, NW, S)))
            nc.vector.tensor_reduce(feats[bkl][:, R_DL:R_DL + 15, 0:NW], dlp[:],
                                    axis=mybir.AxisListType.X,
                                    op=mybir.AluOpType.add)
        # spread (for cov), off the level 0-3 paths
        for bkl in range(NB):
            mean = scratch.tile([128, F, NW], b16, tag="s180")
            nc.vector.tensor_scalar_mul(mean[:], meansums[bkl][:], 1.0 / S)
            spread = spreadp.tile([128, F, NW, S], b16, tag="spread",
                                  name=f"spread{bkl}")
            nc.vector.tensor_sub(
                spread[:], dtiles[bkl][:],
                mean[:, :, :, None].to_broadcast((128, F, NW, S)))
            spreads.append(spread)
        # cov + corr offsets (levels 3+d)
        for dd in range(1, 15):
            for bkl in range(NB):
                nf = 15 - dd
                spread = spreads[bkl]
                feat = feats[bkl]
                rstd = rstds[bkl]
                eng = nc.gpsimd if dd in GPSIMD_DD else nc.vector
                ctag = "gprod" if dd in GPSIMD_DD else "prod"
                cpool = gscratch if dd in GPSIMD_DD else scratch
                cp = cpool.tile([128, nf, NW, S], b16, tag=ctag)
                eng.tensor_mul(cp[:], spread[:, 0:nf], spread[:, dd:15])
                cov_slice = feat[:, R_COV + cb[dd]:R_COV + cb[dd] + nf, 0:NW]
                nc.vector.tensor_reduce(cov_slice, cp[:],
                                        axis=mybir.AxisListType.X,
                                        op=mybir.AluOpType.add)
                rsp = scratch.tile([128, nf, NW], f32, tag="s180f")
                nc.vector.tensor_mul(rsp[:], rstd[:, 0:nf], rstd[:, dd:15])
                nc.vector.tensor_mul(feat[:, cb[dd]:cb[dd] + nf, 0:NW],
                                     cov_slice, rsp[:])

        # ---------------- conv/fc1 stream (PE queue, readiness order) -----
        fc1ps = [ps_fc1.tile([128, NSH], f32, tag=f"jb{jb}", name=f"fc1ps{jb}")
                 for jb in range(4)]

        abatches = {}
        wbatches = {}
        w1chunks = set()

        def get_w1t(tidx):
            c = tidx // 5
            if c not in w1chunks:
                lo, hi = 5 * c, min(5 * (c + 1), NTAIL)
                nc.sync.dma_start(w1t_sb[:, lo:hi, :], w1t_e[:, lo:hi, :])
                w1chunks.add(c)

        def get_ab(s):
            bidx = s // 16
            if bidx not in abatches:
                ab = apool.tile([128, 16, 128], b16, tag="ab", name=f"ab{bidx}")
                nc.sync.dma_start(ab[:], A_e[:, 16 * bidx:16 * (bidx + 1), :])
                abatches[bidx] = ab
            return abatches[bidx], s - 16 * (s // 16)

        def get_wb(ps):
            bidx = ps // 8
            if bidx not in wbatches:
                wb = w8pool.tile([128, 8, 2, 512], f8, tag="wb", name=f"wb{bidx}")
                nc.sync.dma_start(wb[:], w8_e[:, 8 * bidx:8 * (bidx + 1), :, :])
                wbatches[bidx] = wb
            return wbatches[bidx], ps - 8 * (ps // 8)

        def emit_transposes(lvl):
            for u in range(NGT):
                if ulvl[u] != lvl:
                    continue
                for bkl in range(NB):
                    featf = feats[bkl].rearrange("p r w -> p (r w)")
                    tp = ps_conv.tile([128, 128], b16, tag="cps",
                                    name=f"tp{bkl}_{u}")
                    nc.tensor.transpose(tp[:],
                                        featf[:, 128 * u:128 * (u + 1)], ident[:])
                    nc.scalar.activation(
                        f_sb[:, u, 128 * bkl:128 * (bkl + 1)],
                        tp[:], AF.Copy, bias=0.0, scale=1.0)

        def conv_tile(t, aslot, xt_ap, on_dve):
            ab, slot = get_ab(aslot)
            if aslot + 16 < NT:
                get_ab(aslot + 16)            # prefetch next A batch
            cps = ps_conv.tile([128, NSH], f32, tag="cps", name=f"cps{t}")
            nc.tensor.matmul(cps[:], ab[:, slot, :], f_sb[:, piece_refs[t], :],
                             start=True, stop=True)
            if on_dve:
                nc.vector.tensor_scalar(xt_ap, cps[:], yb_sb[:, aslot:aslot + 1],
                                        0.0, op0=mybir.AluOpType.add,
                                        op1=mybir.AluOpType.max)
            else:
                nc.scalar.activation(xt_ap, cps[:], AF.Relu,
                                     bias=yb_sb[:, aslot:aslot + 1], scale=1.0)

        # software pipeline: fc1 matmuls for item k go out after the conv
        # matmuls of item k+2, so PE never waits on an epilogue.
        pending = []
        first_mm = [True] * 4
        n_items = len(items)

        def emit_fc1(entry, is_last):
            kind, buf, meta = entry
            for jb in range(4):
                if kind == 'pair':
                    wb, wslot = meta
                    nc.tensor.matmul(fc1ps[jb][:],
                                     wb[:, wslot, :, 128 * jb:128 * (jb + 1)],
                                     buf[:], start=first_mm[jb], stop=is_last,
                                     perf_mode=DR)
                else:
                    tidx = meta
                    nc.tensor.matmul(fc1ps[jb][:],
                                     w1t_sb[:, tidx, 128 * jb:128 * (jb + 1)],
                                     buf[:], start=first_mm[jb], stop=is_last)
                first_mm[jb] = False

        def flush_pending(is_last, depth=2):
            while pending and (len(pending) > depth or is_last):
                emit_fc1(pending.pop(0), is_last and not pending)

        def emit_item(it, item_idx):
            nonlocal pending
            late = item_idx >= EPI_DVE_FROM
            if it['kind'] == 'pair':
                xt = xtp8.tile([128, 2, 512], f8, tag="xt",
                               name=f"xt{it['P']}")
                for i in range(2):
                    conv_tile(it['tiles'][i], it['aslots'][i], xt[:, i, :],
                              on_dve=(late and i == 1))
                wb, wslot = get_wb(it['pslot'])
                if it['pslot'] + 8 < NPAIR:
                    get_wb(it['pslot'] + 8)   # prefetch next weight batch
                pending.append(('pair', xt, (wb, wslot)))
                flush_pending(False)
            else:
                xtb = xtbp.tile([128, 512], b16, tag="xtb",
                                name=f"xtb{it['tidx']}")
                get_w1t(it['tidx'])
                if it['tidx'] + 5 < NTAIL:
                    get_w1t(it['tidx'] + 5)
                conv_tile(it['tiles'][0], it['aslots'][0], xtb[:],
                          on_dve=(late and it['tidx'] % 2 == 1))
                pending.append(('tail', xtb, it['tidx']))
                flush_pending(False)

        idx = 0
        for lvl in range(NLVL):
            emit_transposes(lvl)
            while idx < n_items and items[idx]['lvl'] == lvl:
                emit_item(items[idx], idx)
                idx += 1
        assert idx == n_items
        flush_pending(True)

        # ---------------- fc1 epilogue / fc2 / fc3 ----------------
        x2 = x2pool.tile([128, 4, NSH], b16, name="x2")
        for jb in range(4):
            if jb % 2:
                nc.vector.tensor_scalar(x2[:, jb, :], fc1ps[jb][:],
                                        1.0 / GQ, None,
                                        op0=mybir.AluOpType.mult)
                nc.vector.tensor_scalar(x2[:, jb, :], x2[:, jb, :],
                                        fc1b_sb[:, jb:jb + 1], 0.0,
                                        op0=mybir.AluOpType.add,
                                        op1=mybir.AluOpType.max)
            else:
                nc.scalar.activation(x2[:, jb, :], fc1ps[jb][:], AF.Relu,
                                     bias=fc1b_sb[:, jb:jb + 1], scale=1.0 / GQ)
        fc2ps = ps_conv.tile([128, NSH], f32, tag="cps")
        for kb in range(4):
            nc.tensor.matmul(fc2ps[:], fc2t_sb[:, kb, :], x2[:, kb, :],
                             start=(kb == 0), stop=(kb == 3))
        x3 = x2pool.tile([128, NSH], b16)
        nc.scalar.activation(x3[:], fc2ps[:], AF.Sigmoid,
                             bias=fc2b_sb[:], scale=1.0)
        fc3ps = ps_fc1.tile([128, NSH], f32, tag="jb1")
        nc.tensor.matmul(fc3ps[0:1, :], fc3t_sb[:], x3[:],
                         start=True, stop=True)
        out_sb = outp.tile([1, NSH], f32)
        nc.scalar.activation(out_sb[:], fc3ps[0:1, :], AF.Identity,
                             bias=fc3b_sb[:], scale=1.0)
        nc.sync.dma_start(out_e[:], out_sb[:])

    nc.compile()
    return nc


# ------------------------------- entry -------------------------------

def _prep_in_maps(inputs):
    dev = _build_device_inputs(inputs)
    dataf = np.asarray(inputs['data'], np.float32).reshape(NFULL, F * W)
    data = np.ascontiguousarray(dataf).astype(bf16)
    data0 = np.ascontiguousarray(
        dataf.reshape(NFULL, F, NW, S)[:, :, :, 0].reshape(NFULL, F * NW))
    shared = {k: dev[k] for k in ('A8', 'w8', 'w1t', 'ybias2d', 'fc1b2d',
                                  'fc2t', 'fc2b', 'fc3t', 'fc3b', 'wdl')}
    in_maps = []
    for c in range(NCORES):
        m = dict(shared)
        m['data'] = data[NSH * c:NSH * (c + 1)]
        m['data0'] = data0[NSH * c:NSH * (c + 1)]
        in_maps.append(m)
    return dev, in_maps


def run(inputs, trace=False, tmpdir=None):
    dev, in_maps = _prep_in_maps(inputs)
    nc = build_nc(dev['piece_refs'], dev['ulvl'], dev['items'])
    res = run_bass_kernel_spmd(nc, in_maps, core_ids=list(range(NCORES)),
                               trace=trace, tmpdir=tmpdir)
    out = np.concatenate([np.asarray(r["out"], np.float32).reshape(NSH)
                          for r in res.results])
    return out, res


def kernel(**inputs) -> np.ndarray:
    out, _ = run(inputs, trace=False)
    return out


# revision 47
# speedup vs baseline: 1.0297x; 1.0297x over previous
"""AlphaNet forward pass on 8 Trainium2 NeuronCores (data-parallel over batch).

Pipeline per core (512 samples):
  DVE: rolling-window stats (corr/cov/std/zscore/return/decaylinear) in an
       unscaled "mine" form, written into a padded [272 rows x 16 win] bf16
       buffer. Mine row order: corr(0:105) cov(105:210) std(210:225)
       zs(225:240) dl(240:255) dummy(255) ret(256:271) -- ret rows last so
       the fp8 fc1 region excludes the huge-dynamic-range return rows,
       which stay in a bf16 tail.
  PE : transpose features to [rows, samples] (bf16); conv(1x3)+BN folded
       into per-tile A matmuls; fc1 as fp8-e4m3 DoubleRow matmuls (K=256
       per instruction) plus a bf16 tail; fc2/fc3 bf16. fc1 weights are
       pre-scaled by GQ=4096 into e4m3's normal range; the fc1 relu
       epilogue divides by GQ.
  Overlap: stats are computed pair-offset-major (d=1..14 across all four
       sample blocks), and conv/fc1 tile pairs are issued in row-readiness
       order so the PE stream starts right after the cheap simple stats
       and runs concurrently with the remaining DVE work. Scalar engine
       owns the conv epilogues (bias+relu -> fp8/bf16); GpSimd takes
       memsets and the d=1 cov products. fc1 matmuls for pair k are issued
       after the conv matmuls of pair k+1 so PE never stalls on epilogues.
All per-row constant factors (BatchNorm affine, 1/9, 0.9, 0.3, ret's -1, ...)
are folded into the host-built conv matrix A and per-row bias. Weights are
host-permuted/packed in issue order; DMAs are batched (16 A-tiles / 8 fc1
weight pairs per transfer) from partition-major DRAM layouts.
"""
import sys
for _p in ("/opt/trn_rl_repo", "/root/.axon_site/_ro/trn_rl_repo"):
    if _p not in sys.path:
        sys.path.append(_p)

from contextlib import ExitStack

import numpy as np
import ml_dtypes

import concourse.bass as bass
import concourse.tile as tile
from concourse import bacc, mybir
from concourse.bass_utils import run_bass_kernel_spmd
from concourse.masks import make_identity

bf16 = ml_dtypes.bfloat16
f8e4 = ml_dtypes.float8_e4m3
dt = mybir.dt

# ---- problem constants (hardcoded; must match the AlphaNet reference) ----
NFULL = 4096
NCORES = 8
NSH = NFULL // NCORES        # 512 samples per core
F, W, S = 15, 120, 10
NW = W // S                  # 12
HP = 270                     # reference stat rows
HPM = 271                    # mine rows incl dummy row 255
NROW_PAD, WPAD = 272, 16
GROWS = NROW_PAD * WPAD      # 4352 = 34*128
NGT = GROWS // 128           # 34 transposed-feature tiles
K1M = HPM * 160              # 43360 kernel-order K
NT = 339                     # ceil(K1M/128)
K1PAD = NT * 128             # 43392
NT8 = 320                    # fp8 tiles (pairs 0:160); bf16 tail 320:339
NPAIR = NT8 // 2             # 160 DoubleRow pairs
NTAIL = NT - NT8             # 19
NTA = 352                    # A tile slots padded to 22*16 for uniform batches
BN_EPS = 1e-5
NB = NSH // 128              # 4 sample blocks per core
GQ = 4096.0                  # global fc1 weight scale (fp8 normal range)
GPSIMD_DD = ()               # cov products on GpSimd contend for DVE's SBUF ports
NLVL = 18                    # readiness levels: 0=std 1=ret 2=zs 3=dl 3+d=cov/corr d
EPI_DVE_FROM = 108           # items from this index split epilogues Scalar/DVE

# mine row blocks
R_CORR, R_COV, R_STD, R_ZS, R_DL, R_DUM, R_RET = 0, 105, 210, 225, 240, 255, 256


# ------------------------- host-side preparation -------------------------

def _mine_row_tables():
    cb, pairs, base = {}, [], 0
    for d in range(1, 15):
        cb[d] = base
        for i in range(0, 15 - d):
            pairs.append((i, i + d))
        base += 15 - d
    return pairs, cb


def _ref_perm():
    """rom[mine_row] = reference feature row (or -1 for the dummy row)."""
    pairs, _ = _mine_row_tables()
    II, JJ = np.triu_indices(F, k=1)
    p2r = {(int(i), int(j)): p for p, (i, j) in enumerate(zip(II, JJ))}
    rom = np.full(HPM, -1, dtype=np.int64)
    for mh, (i, j) in enumerate(pairs):
        rom[mh] = p2r[(i, j)]
        rom[R_COV + mh] = 105 + p2r[(i, j)]
    for i in range(15):
        rom[R_STD + i] = 210 + i
        rom[R_ZS + i] = 225 + i
        rom[R_DL + i] = 255 + i     # reference decaylinear block
        rom[R_RET + i] = 240 + i    # reference return block
    return rom


def _row_alpha_beta():
    """mine = alpha*ref + beta per row."""
    alpha = np.ones(HPM)
    beta = np.zeros(HPM)
    alpha[R_CORR:R_COV] = 1.0 / 0.9
    alpha[R_COV:R_STD] = 9.0
    alpha[R_STD:R_ZS] = 3.0
    alpha[R_ZS:R_DL] = 10.0 / 3.0
    alpha[R_DL:R_DUM] = 1.0
    alpha[R_RET:] = 1.0
    beta[R_RET:] = 1.0
    return alpha, beta


def _piece_refs():
    refs = []
    for t in range(NT):
        h0 = (128 * t) // 160
        hl = min((128 * t + 127) // 160, HPM - 1)
        u = h0 // 8
        assert all(8 * u <= h < 8 * u + 8 for h in range(h0, hl + 1)), (t, h0, hl)
        refs.append(u)
    return refs


def _schedule(refs):
    """Row-readiness schedule. Returns (ulvl, items) where items is the
    conv/fc1 issue order: dicts {kind, lvl, tiles, pslot or tidx, aslots}.
    lvl is the stats level after which the item's rows are complete:
    0=std 1=zscore 2=decaylinear 3=return 3+d=cov/corr offset d."""
    _, cb = _mine_row_tables()

    def lvl_row(r):
        if r >= R_RET:
            return 0        # return rows unlock the whole bf16 tail first
        if R_STD <= r < R_ZS:
            return 1
        if R_ZS <= r < R_DL:
            return 2
        if R_DL <= r < R_DUM:
            return 3
        c = r if r < 105 else r - 105
        for d in range(1, 15):
            if c < cb[d] + (15 - d):
                return 3 + d
        raise AssertionError(r)

    ulvl = []
    for u in range(NGT):
        m = 0
        for r in range(8 * u, 8 * u + 8):
            if r < HPM and r != R_DUM:
                m = max(m, lvl_row(r))
        ulvl.append(m)

    pair_lvl = [max(ulvl[refs[2 * P]], ulvl[refs[2 * P + 1]])
                for P in range(NPAIR)]
    items = []
    for lvl in range(NLVL):
        for P in range(NPAIR):
            if pair_lvl[P] == lvl:
                items.append(dict(kind='pair', lvl=lvl, P=P,
                                  tiles=[2 * P, 2 * P + 1]))
        if lvl == 0:
            for t in range(NT8, NT):
                assert ulvl[refs[t]] == 0
                items.append(dict(kind='tail', lvl=0, tiles=[t],
                                  tidx=t - NT8))
    # assign A slots and fp8 pair slots in issue order
    aslot = 0
    pslot = 0
    for it in items:
        it['aslots'] = list(range(aslot, aslot + len(it['tiles'])))
        aslot += len(it['tiles'])
        if it['kind'] == 'pair':
            it['pslot'] = pslot
            pslot += 1
    assert aslot == NT and pslot == NPAIR
    return ulvl, items


def _build_device_inputs(inp):
    gamma = float(inp['bn_gamma'][0]); betab = float(inp['bn_beta'][0])
    mu = float(inp['bn_mean'][0]); var = float(inp['bn_var'][0])
    a = gamma / np.sqrt(var + BN_EPS)
    b = betab - mu * a
    conv_w = np.asarray(inp['conv_w'], np.float64).reshape(16, 3)
    conv_b = np.asarray(inp['conv_b'], np.float64)

    alpha, beta = _row_alpha_beta()
    sA = a / alpha
    sB = b - a * beta / alpha

    refs = _piece_refs()
    ulvl, items = _schedule(refs)

    wsum = conv_w.sum(axis=1)
    ybias = np.zeros(K1PAD, np.float64)
    for mh in range(HPM):
        if mh == R_DUM:
            continue
        ybias[mh * 160:(mh + 1) * 160] = np.repeat(conv_b + wsum * sB[mh], 10)

    # fc1 weights: permute reference K to kernel (mh, o, wp) order, scale GQ
    rom = _ref_perm()
    m = np.arange(K1M)
    mh = m // 160; o = (m % 160) // 10; wp = m % 10
    valid = mh != R_DUM
    kref = o * 2700 + rom[mh] * 10 + wp
    fc1_w = np.asarray(inp['fc1_w'], np.float32)
    fc1t = np.zeros((K1PAD, 512), np.float32)
    fc1t[m[valid], :] = fc1_w[:, kref[valid]].T * GQ

    # conv A per tile [t, 128(k), 128(col)]
    A_t = np.zeros((NT, 128, 128), np.float32)
    for t in range(NT):
        u = refs[t]
        for col in range(128):
            mm = 128 * t + col
            if mm >= K1M:
                continue
            mhh = mm // 160
            if mhh == R_DUM:
                continue
            oo = (mm % 160) // 10
            wpp = mm % 10
            for k in range(3):
                A_t[t, (mhh - 8 * u) * 16 + wpp + k, col] = conv_w[oo, k] * sA[mhh]

    # pack A / ybias by A slot and fc1 fp8 pairs by pair slot (issue order)
    A8 = np.zeros((128, NTA, 128), np.float32)
    yb2d = np.zeros((128, NTA), np.float32)
    w8 = np.zeros((128, NPAIR, 2, 512), np.float32)
    for it in items:
        for t, s in zip(it['tiles'], it['aslots']):
            A8[:, s, :] = A_t[t]
            yb2d[:, s] = ybias[128 * t:128 * (t + 1)]
        if it['kind'] == 'pair':
            for i in range(2):
                t = it['tiles'][i]
                w8[:, it['pslot'], i, :] = fc1t[128 * t:128 * (t + 1), :]
    assert np.abs(w8).max() < 200.0, np.abs(w8).max()
    A8 = A8.astype(bf16)
    w8 = w8.astype(f8e4)
    # bf16 tail weights by tidx: [128(k), NTAIL, 512]
    w1t = np.ascontiguousarray(
        fc1t[NT8 * 128:].reshape(NTAIL, 128, 512).transpose(1, 0, 2)).astype(bf16)

    fc1b2d = np.asarray(inp['fc1_b'], np.float32).reshape(4, 128).T.copy()
    fc2t = np.ascontiguousarray(
        np.asarray(inp['fc2_w'], np.float32).T).astype(bf16)
    fc2b = np.asarray(inp['fc2_b'], np.float32).reshape(128, 1).copy()
    fc3t = np.ascontiguousarray(
        np.asarray(inp['fc3_w'], np.float32).reshape(1, 128).T).astype(bf16)
    fc3b = np.asarray(inp['fc3_b'], np.float32).reshape(1, 1).copy()
    wdl = np.tile((np.arange(1, 11, dtype=np.float32) / 55.0)[None, :],
                  (128, 1)).astype(bf16)

    return dict(A8=A8, w8=w8, w1t=w1t, piece_refs=refs,
                ulvl=ulvl, items=items,
                ybias2d=yb2d, fc1b2d=fc1b2d,
                fc2t=fc2t, fc2b=fc2b, fc3t=fc3t, fc3b=fc3b, wdl=wdl)


# ------------------------- device kernel builder -------------------------

def build_nc(piece_refs, ulvl, items):
    nc = bacc.Bacc("TRN2", target_bir_lowering=False, debug=False,
                   num_devices=NCORES)
    f32, b16, f8 = dt.float32, dt.bfloat16, dt.float8e4
    data_e = nc.declare_dram_parameter("data", [NSH, F * W], b16, isOutput=False)
    data0_e = nc.declare_dram_parameter("data0", [NSH, F * NW], f32, isOutput=False)
    A_e = nc.declare_dram_parameter("A8", [128, NTA, 128], b16, isOutput=False)
    w8_e = nc.declare_dram_parameter("w8", [128, NPAIR, 2, 512], f8, isOutput=False)
    w1t_e = nc.declare_dram_parameter("w1t", [128, NTAIL, 512], b16, isOutput=False)
    yb_e = nc.declare_dram_parameter("ybias2d", [128, NTA], f32, isOutput=False)
    fc1b_e = nc.declare_dram_parameter("fc1b2d", [128, 4], f32, isOutput=False)
    fc2t_e = nc.declare_dram_parameter("fc2t", [512, 128], b16, isOutput=False)
    fc2b_e = nc.declare_dram_parameter("fc2b", [128, 1], f32, isOutput=False)
    fc3t_e = nc.declare_dram_parameter("fc3t", [128, 1], b16, isOutput=False)
    fc3b_e = nc.declare_dram_parameter("fc3b", [1, 1], f32, isOutput=False)
    wdl_e = nc.declare_dram_parameter("wdl", [128, 10], b16, isOutput=False)
    out_e = nc.declare_dram_parameter("out", [1, NSH], f32, isOutput=True)

    _, cb = _mine_row_tables()
    AF = mybir.ActivationFunctionType
    DR = mybir.MatmulPerfMode.DoubleRow

    with tile.TileContext(nc) as tc, ExitStack() as ctx:
        consts = ctx.enter_context(tc.tile_pool(name="consts", bufs=1))
        fpool = ctx.enter_context(tc.tile_pool(name="fpool", bufs=1))
        datap = ctx.enter_context(tc.tile_pool(name="datap", bufs=4))
        featp = ctx.enter_context(tc.tile_pool(name="featp", bufs=4))
        spreadp = ctx.enter_context(tc.tile_pool(name="spreadp", bufs=4))
        meansump = ctx.enter_context(tc.tile_pool(name="meansump", bufs=4))
        rstdp = ctx.enter_context(tc.tile_pool(name="rstdp", bufs=4))
        scratch = ctx.enter_context(tc.tile_pool(name="scratch", bufs=4))
        gscratch = ctx.enter_context(tc.tile_pool(name="gscratch", bufs=2))
        xtp8 = ctx.enter_context(tc.tile_pool(name="xtp8", bufs=4))
        xtbp = ctx.enter_context(tc.tile_pool(name="xtbp", bufs=4))
        w8pool = ctx.enter_context(tc.tile_pool(name="w8pool", bufs=3))
        apool = ctx.enter_context(tc.tile_pool(name="apool", bufs=3))
        x2pool = ctx.enter_context(tc.tile_pool(name="x2pool", bufs=1))
        outp = ctx.enter_context(tc.tile_pool(name="outp", bufs=1))
        ps_fc1 = ctx.enter_context(tc.tile_pool(name="ps_fc1", bufs=1, space="PSUM"))
        ps_conv = ctx.enter_context(tc.tile_pool(name="ps_conv", bufs=4, space="PSUM"))

        lp = ctx.enter_context(nc.allow_low_precision(
            reason="stat rows feed a bf16 feature buffer; DVE reduces "
                   "accumulate in f32 internally, only the store is bf16"))

        # data DMAs first so stats start immediately
        dtiles = []
        d0tiles = []
        for bkl in range(NB):
            d = datap.tile([128, F, NW, S], b16, tag="d", name=f"d{bkl}")
            nc.sync.dma_start(
                d[:], data_e[128 * bkl:128 * (bkl + 1), :]
                .rearrange("p (f nw s) -> p f nw s", f=F, nw=NW))
            dtiles.append(d)
            d0 = datap.tile([128, F, NW], f32, tag="d0", name=f"d0{bkl}")
            nc.sync.dma_start(
                d0[:], data0_e[128 * bkl:128 * (bkl + 1), :]
                .rearrange("p (f nw) -> p f nw", f=F))
            d0tiles.append(d0)

        # constants
        yb_sb = consts.tile([128, NTA], f32)
        nc.sync.dma_start(yb_sb[:], yb_e[:])
        fc1b_sb = consts.tile([128, 4], f32)
        nc.sync.dma_start(fc1b_sb[:], fc1b_e[:])
        fc2t_sb = consts.tile([128, 4, 128], b16)
        nc.sync.dma_start(fc2t_sb[:], fc2t_e.rearrange("(kb k) j -> k kb j", k=128))
        fc2b_sb = consts.tile([128, 1], f32)
        nc.sync.dma_start(fc2b_sb[:], fc2b_e[:])
        fc3t_sb = consts.tile([128, 1], b16)
        nc.sync.dma_start(fc3t_sb[:], fc3t_e[:])
        fc3b_sb = consts.tile([1, 1], f32)
        nc.sync.dma_start(fc3b_sb[:], fc3b_e[:])
        wdl_sb = consts.tile([128, 10], b16)
        nc.sync.dma_start(wdl_sb[:], wdl_e[:])
        w1t_sb = consts.tile([128, NTAIL, 512], b16)
        ident = consts.tile([128, 128], b16)
        make_identity(nc, ident[:])

        # dummy matmuls to burn through the HAM half-clock warmup window
        # while DVE computes the head stats (PE would otherwise start cold)
        for wu in range(32):
            tpw = ps_conv.tile([128, 128], b16, tag="cps", name=f"warm{wu}")
            nc.tensor.transpose(tpw[:], ident[:], ident[:])

        # persistent bf16 transposed-feature buffer [row, sample]
        f_sb = fpool.tile([128, NGT, NSH], b16)

        feats, spreads, rstds, meansums, varsums = [], [], [], [], []

        # ---------------- stats emission (DVE/GpSimd/Scalar queues) -------
        # Ordered so conv/fc1 work unlocks as early as possible: per-block
        # core (spread/var/std) first, then zscore, decaylinear, return,
        # then cov/corr offsets d=1..14. The conv/fc1 stream below is
        # emitted to independent engine queues; cross-engine semaphores
        # enforce the data dependencies.
        # padding memsets up front on GpSimd, while the data DMAs are in
        # flight (they only gate the transposes, not the stats)
        for bkl in range(NB):
            feat = featp.tile([128, NROW_PAD, WPAD], b16, tag="feat",
                              name=f"feat{bkl}")
            # zero only the padding (w cols 12:16, dummy row 255, pad row 271)
            nc.gpsimd.memset(feat[:, :, NW:WPAD], 0.0)
            nc.gpsimd.memset(feat[:, R_DUM:R_DUM + 1, 0:NW], 0.0)
            nc.gpsimd.memset(feat[:, HPM:NROW_PAD, 0:NW], 0.0)
            feats.append(feat)

        # level 0: return rows (last/first) straight off the raw data --
        # they unlock the whole bf16 tail, giving PE work while the std
        # chain below still runs. level 1: std via
        # sigma^2 = sum(d^2) - (sum d)^2/S, so neither mean/spread nor any
        # reciprocal gates the early tiles.
        with tc.high_priority():
            for bkl in range(NB):
                recipf = scratch.tile([128, F, NW], f32, tag="s180f")
                nc.vector.reciprocal_approx_fast(recipf[:], d0tiles[bkl][:])
                nc.vector.tensor_mul(feats[bkl][:, R_RET:R_RET + 15, 0:NW],
                                     dtiles[bkl][:, :, :, S - 1], recipf[:])
            for bkl in range(NB):
                d = dtiles[bkl]
                feat = feats[bkl]
                meansum = meansump.tile([128, F, NW], f32, tag="ms",
                                        name=f"ms{bkl}")
                nc.vector.tensor_reduce(meansum[:], d[:],
                                        axis=mybir.AxisListType.X,
                                        op=mybir.AluOpType.add)
                ddp = scratch.tile([128, F, NW, S], b16, tag="prod")
                nc.vector.tensor_mul(ddp[:], d[:], d[:])
                ddsum = scratch.tile([128, F, NW], f32, tag="s180f")
                nc.vector.tensor_reduce(ddsum[:], ddp[:],
                                        axis=mybir.AxisListType.X,
                                        op=mybir.AluOpType.add)
                msq = scratch.tile([128, F, NW], f32, tag="s180g")
                nc.vector.tensor_mul(msq[:], meansum[:], meansum[:])
                varsum = scratch.tile([128, F, NW], f32, tag="vsum")
                nc.vector.scalar_tensor_tensor(varsum[:], msq[:], -1.0 / S,
                                               ddsum[:],
                                               op0=mybir.AluOpType.mult,
                                               op1=mybir.AluOpType.add)
                nc.scalar.activation(feat[:, R_STD:R_STD + 15, 0:NW], varsum[:],
                                     AF.Sqrt, bias=0.0, scale=1.0)
                varsums.append(varsum)
                meansums.append(meansum)
        # zscore (level 2); rstd = sqrt(1/varsum), sqrt on Scalar
        for bkl in range(NB):
            rvar = scratch.tile([128, F, NW], f32, tag="s180g")
            nc.vector.reciprocal_approx_fast(rvar[:], varsums[bkl][:])
            rstd = rstdp.tile([128, F, NW], f32, tag="rstd", name=f"rstd{bkl}")
            nc.scalar.activation(rstd[:], rvar[:], AF.Sqrt,
                                 bias=0.0, scale=1.0)
            rstds.append(rstd)
            nc.vector.tensor_mul(feats[bkl][:, R_ZS:R_ZS + 15, 0:NW],
                                 meansums[bkl][:], rstd[:])
        # decay-linear (level 3, mine rows 240:255)
        for bkl in range(NB):
            dlp = scratch.tile([128, F, NW, S], b16, tag="prod")
            nc.vector.tensor_mul(
                dlp[:], dtiles[bkl][:],
                wdl_sb[:, None, None, :].to_broadcast((128, F, NW, S)))
            nc.vector.tensor_reduce(feats[bkl][:, R_DL:R_DL + 15, 0:NW], dlp[:],
                                    axis=mybir.AxisListType.X,
                                    op=mybir.AluOpType.add)
        # spread (for cov), off the level 0-3 paths
        for bkl in range(NB):
            mean = scratch.tile([128, F, NW], b16, tag="s180")
            nc.vector.tensor_scalar_mul(mean[:], meansums[bkl][:], 1.0 / S)
            spread = spreadp.tile([128, F, NW, S], b16, tag="spread",
                                  name=f"spread{bkl}")
            nc.vector.tensor_sub(
                spread[:], dtiles[bkl][:],
                mean[:, :, :, None].to_broadcast((128, F, NW, S)))
            spreads.append(spread)
        # cov + corr offsets (levels 3+d)
        for dd in range(1, 15):
            for bkl in range(NB):
                nf = 15 - dd
                spread = spreads[bkl]
                feat = feats[bkl]
                rstd = rstds[bkl]
                eng = nc.gpsimd if dd in GPSIMD_DD else nc.vector
                ctag = "gprod" if dd in GPSIMD_DD else "prod"
                cpool = gscratch if dd in GPSIMD_DD else scratch
                cp = cpool.tile([128, nf, NW, S], b16, tag=ctag)
                eng.tensor_mul(cp[:], spread[:, 0:nf], spread[:, dd:15])
                cov_slice = feat[:, R_COV + cb[dd]:R_COV + cb[dd] + nf, 0:NW]
                nc.vector.tensor_reduce(cov_slice, cp[:],
                                        axis=mybir.AxisListType.X,
                                        op=mybir.AluOpType.add)
                rsp = scratch.tile([128, nf, NW], f32, tag="s180f")
                nc.vector.tensor_mul(rsp[:], rstd[:, 0:nf], rstd[:, dd:15])
                nc.vector.tensor_mul(feat[:, cb[dd]:cb[dd] + nf, 0:NW],
                                     cov_slice, rsp[:])

        # ---------------- conv/fc1 stream (PE queue, readiness order) -----
        fc1ps = [ps_fc1.tile([128, NSH], f32, tag=f"jb{jb}", name=f"fc1ps{jb}")
                 for jb in range(4)]

        abatches = {}
        wbatches = {}
        w1chunks = set()

        def get_w1t(tidx):
            c = tidx // 5
            if c not in w1chunks:
                lo, hi = 5 * c, min(5 * (c + 1), NTAIL)
                nc.sync.dma_start(w1t_sb[:, lo:hi, :], w1t_e[:, lo:hi, :])
                w1chunks.add(c)

        def get_ab(s):
            bidx = s // 16
            if bidx not in abatches:
                ab = apool.tile([128, 16, 128], b16, tag="ab", name=f"ab{bidx}")
                nc.sync.dma_start(ab[:], A_e[:, 16 * bidx:16 * (bidx + 1), :])
                abatches[bidx] = ab
            return abatches[bidx], s - 16 * (s // 16)

        def get_wb(ps):
            bidx = ps // 8
            if bidx not in wbatches:
                wb = w8pool.tile([128, 8, 2, 512], f8, tag="wb", name=f"wb{bidx}")
                nc.sync.dma_start(wb[:], w8_e[:, 8 * bidx:8 * (bidx + 1), :, :])
                wbatches[bidx] = wb
            return wbatches[bidx], ps - 8 * (ps // 8)

        def emit_transposes(lvl):
            for u in range(NGT):
                if ulvl[u] != lvl:
                    continue
                for bkl in range(NB):
                    featf = feats[bkl].rearrange("p r w -> p (r w)")
                    tp = ps_conv.tile([128, 128], b16, tag="cps",
                                    name=f"tp{bkl}_{u}")
                    nc.tensor.transpose(tp[:],
                                        featf[:, 128 * u:128 * (u + 1)], ident[:])
                    nc.scalar.activation(
                        f_sb[:, u, 128 * bkl:128 * (bkl + 1)],
                        tp[:], AF.Copy, bias=0.0, scale=1.0)

        def conv_tile(t, aslot, xt_ap, on_dve):
            ab, slot = get_ab(aslot)
            if aslot + 16 < NT:
                get_ab(aslot + 16)            # prefetch next A batch
            cps = ps_conv.tile([128, NSH], f32, tag="cps", name=f"cps{t}")
            nc.tensor.matmul(cps[:], ab[:, slot, :], f_sb[:, piece_refs[t], :],
                             start=True, stop=True)
            if on_dve:
                nc.vector.tensor_scalar(xt_ap, cps[:], yb_sb[:, aslot:aslot + 1],
                                        0.0, op0=mybir.AluOpType.add,
                                        op1=mybir.AluOpType.max)
            else:
                nc.scalar.activation(xt_ap, cps[:], AF.Relu,
                                     bias=yb_sb[:, aslot:aslot + 1], scale=1.0)

        # software pipeline: fc1 matmuls for item k go out after the conv
        # matmuls of item k+2, so PE never waits on an epilogue.
        pending = []
        first_mm = [True] * 4
        n_items = len(items)

        def emit_fc1(entry, is_last):
            kind, buf, meta = entry
            for jb in range(4):
                if kind == 'pair':
                    wb, wslot = meta
                    nc.tensor.matmul(fc1ps[jb][:],
                                     wb[:, wslot, :, 128 * jb:128 * (jb + 1)],
                                     buf[:], start=first_mm[jb], stop=is_last,
                                     perf_mode=DR)
                else:
                    tidx = meta
                    nc.tensor.matmul(fc1ps[jb][:],
                                     w1t_sb[:, tidx, 128 * jb:128 * (jb + 1)],
                                     buf[:], start=first_mm[jb], stop=is_last)
                first_mm[jb] = False

        def flush_pending(is_last, depth=2):
            while pending and (len(pending) > depth or is_last):
                emit_fc1(pending.pop(0), is_last and not pending)

        def emit_item(it, item_idx):
            nonlocal pending
            late = item_idx >= EPI_DVE_FROM
            if it['kind'] == 'pair':
                xt = xtp8.tile([128, 2, 512], f8, tag="xt",
                               name=f"xt{it['P']}")
                for i in range(2):
                    conv_tile(it['tiles'][i], it['aslots'][i], xt[:, i, :],
                              on_dve=(late and i == 1))
                wb, wslot = get_wb(it['pslot'])
                if it['pslot'] + 8 < NPAIR:
                    get_wb(it['pslot'] + 8)   # prefetch next weight batch
                pending.append(('pair', xt, (wb, wslot)))
                flush_pending(False)
            else:
                xtb = xtbp.tile([128, 512], b16, tag="xtb",
                                name=f"xtb{it['tidx']}")
                get_w1t(it['tidx'])
                if it['tidx'] + 5 < NTAIL:
                    get_w1t(it['tidx'] + 5)
                conv_tile(it['tiles'][0], it['aslots'][0], xtb[:],
                          on_dve=(late and it['tidx'] % 2 == 1))
                pending.append(('tail', xtb, it['tidx']))
                flush_pending(False)

        idx = 0
        for lvl in range(NLVL):
            emit_transposes(lvl)
            while idx < n_items and items[idx]['lvl'] == lvl:
                emit_item(items[idx], idx)
                idx += 1
        assert idx == n_items
        flush_pending(True)

        # ---------------- fc1 epilogue / fc2 / fc3 ----------------
        x2 = x2pool.tile([128, 4, NSH], b16, name="x2")
        for jb in range(4):
            if jb % 2:
                nc.vector.tensor_scalar(x2[:, jb, :], fc1ps[jb][:],
                                        1.0 / GQ, None,
                                        op0=mybir.AluOpType.mult)
                nc.vector.tensor_scalar(x2[:, jb, :], x2[:, jb, :],
                                        fc1b_sb[:, jb:jb + 1], 0.0,
                                        op0=mybir.AluOpType.add,
                                        op1=mybir.AluOpType.max)
            else:
                nc.scalar.activation(x2[:, jb, :], fc1ps[jb][:], AF.Relu,
                                     bias=fc1b_sb[:, jb:jb + 1], scale=1.0 / GQ)
        fc2ps = ps_conv.tile([128, NSH], f32, tag="cps")
        for kb in range(4):
            nc.tensor.matmul(fc2ps[:], fc2t_sb[:, kb, :], x2[:, kb, :],
                             start=(kb == 0), stop=(kb == 3))
        x3 = x2pool.tile([128, NSH], b16)
        nc.scalar.activation(x3[:], fc2ps[:], AF.Sigmoid,
                             bias=fc2b_sb[:], scale=1.0)
        fc3ps = ps_fc1.tile([128, NSH], f32, tag="jb1")
        nc.tensor.matmul(fc3ps[0:1, :], fc3t_sb[:], x3[:],
                         start=True, stop=True)
        out_sb = outp.tile([1, NSH], f32)
        nc.scalar.activation(out_sb[:], fc3ps[0:1, :], AF.Identity,
                             bias=fc3b_sb[:], scale=1.0)
        nc.sync.dma_start(out_e[:], out_sb[:])

    nc.compile()
    return nc


# ------------------------------- entry -------------------------------

def _prep_in_maps(inputs):
    dev = _build_device_inputs(inputs)
    dataf = np.asarray(inputs['data'], np.float32).reshape(NFULL, F * W)
    data = np.ascontiguousarray(dataf).astype(bf16)
    data0 = np.ascontiguousarray(
        dataf.reshape(NFULL, F, NW, S)[:, :, :, 0].reshape(NFULL, F * NW))
    shared = {k: dev[k] for k in ('A8', 'w8', 'w1t', 'ybias2d', 'fc1b2d',
                                  'fc2t', 'fc2b', 'fc3t', 'fc3b', 'wdl')}
    in_maps = []
    for c in range(NCORES):
        m = dict(shared)
        m['data'] = data[NSH * c:NSH * (c + 1)]
        m['data0'] = data0[NSH * c:NSH * (c + 1)]
        in_maps.append(m)
    return dev, in_maps


def run(inputs, trace=False, tmpdir=None):
    dev, in_maps = _prep_in_maps(inputs)
    nc = build_nc(dev['piece_refs'], dev['ulvl'], dev['items'])
    res = run_bass_kernel_spmd(nc, in_maps, core_ids=list(range(NCORES)),
                               trace=trace, tmpdir=tmpdir)
    out = np.concatenate([np.asarray(r["out"], np.float32).reshape(NSH)
                          for r in res.results])
    return out, res


def kernel(**inputs) -> np.ndarray:
    out, _ = run(inputs, trace=False)
    return out
